# revision 1
# baseline (speedup 1.0000x reference)
"""Trainium2 Bass kernel for nn_CLLayer (SimCLR-style contrastive loss).

Math (reference, tau=0.5):
    h1 = elu(z1 @ W1.T + b1) @ W2.T + b2 ; h2 likewise
    n1, n2 = row-normalized h1, h2
    l1_i = log(sum_j exp(2*n1_i.n1_j) + sum_j exp(2*n1_i.n2_j) - e^2) - 2*n1_i.n2_i
    l2_i = log(sum_j exp(2*n2_i.n2_j) + sum_j exp(2*n2_j.n1_i... ) - e^2) - 2*...
    out = 0.5*(l1+l2)

Sharding: row-parallel over N=8192 (1024 rows/core, 8 cores).
Each core: projects its row block (bf16 matmuls), normalizes, AllGathers
normalized embeddings (bf16), computes its row-strip of the three distinct
similarity products (S12, S22, S11), exp+row-sums on the fly, column-sums of
exp(2*S12) via a ReduceScatter (between2 = between.T so l2's "between" row
sums are column sums of S12's exp).  Only 3 of 4 N^2*D products are needed.

Host-side prep: transposes z blocks / weights to K-major (PE wants K on
partitions), casts matmul operands to bf16, and folds the ELU "-1" into an
adjusted fc2 bias (b2' = b2 - fc2_w.sum(1)) so ELU is computed as
relu(x) + exp(min(x,0)) without the subtract (device ELU' = elu + 1).
"""

import math
import os
from functools import lru_cache

import ml_dtypes
import numpy as np

import concourse.bacc as bacc
import concourse.bass as bass
import concourse.mybir as mybir
import concourse.tile as tile
from concourse.bass_utils import run_bass_kernel_spmd

N, D = 8192, 1024
NCORES = 8
BLK = N // NCORES  # 1024
P = 128
KO = D // P  # 8 k-tiles
NT = BLK // P  # 8 i-tiles per core
JC = N // 512  # 16 j-chunks of 512
E2 = float(np.exp(2.0))  # exp(1/tau), tau=0.5
BF = mybir.dt.bfloat16
F32 = mybir.dt.float32
AF = mybir.ActivationFunctionType
ALU = mybir.AluOpType


def _build():
    nc = bacc.Bacc("TRN2", target_bir_lowering=False, debug=False, num_devices=NCORES)

    z1t = nc.dram_tensor("z1t", [D, BLK], BF, kind="ExternalInput")
    z2t = nc.dram_tensor("z2t", [D, BLK], BF, kind="ExternalInput")
    w1t = nc.dram_tensor("w1t", [D, D], BF, kind="ExternalInput")
    w2t = nc.dram_tensor("w2t", [D, D], BF, kind="ExternalInput")
    b1 = nc.dram_tensor("b1", [D], F32, kind="ExternalInput")
    b2p = nc.dram_tensor("b2p", [D], F32, kind="ExternalInput")
    out = nc.dram_tensor("out", [BLK], F32, kind="ExternalOutput")

    kp = lambda ap: ap.rearrange("(ko ki) x -> ki ko x", ki=P)  # K-major -> [128, KO, x]
    pt = lambda ap: ap.rearrange("(t p) -> p t", p=P)  # [1024] -> [128, 8]
    JP = JC // 2  # 8 j-chunk-pairs of 1024

    with tile.TileContext(nc) as tc:
        with (
            tc.tile_pool(name="consts", bufs=1) as consts,
            tc.tile_pool(name="mats", bufs=1) as mats,
            tc.tile_pool(name="strip", bufs=1) as strip,
            tc.tile_pool(name="scratch", bufs=2) as scratch,
            tc.tile_pool(name="rhs", bufs=3) as rhsp,
            tc.tile_pool(name="expp", bufs=2) as expp,
            tc.tile_pool(name="small", bufs=1) as small,
            tc.tile_pool(name="psA", bufs=3, space="PSUM") as psA,
            tc.tile_pool(name="psB", bufs=2, space="PSUM") as psB,
            tc.tile_pool(name="dram", bufs=1, space="DRAM") as dram,
        ):
            # ---------------- constants ----------------
            w1_sb = consts.tile([P, KO, D], BF)
            w2_sb = consts.tile([P, KO, D], BF)
            nc.sync.dma_start(w1_sb[:], kp(w1t[:]))
            nc.sync.dma_start(w2_sb[:], kp(w2t[:]))
            b1_sb = consts.tile([P, KO], F32)
            b2_sb = consts.tile([P, KO], F32)
            nc.sync.dma_start(b1_sb[:], pt(b1[:]))
            nc.sync.dma_start(b2_sb[:], pt(b2p[:]))
            ones_bf = consts.tile([P, 1], BF)
            ones_f = consts.tile([P, 1], F32)
            nc.vector.memset(ones_bf[:], 1.0)
            nc.vector.memset(ones_f[:], 1.0)

            z_sb = mats.tile([P, KO, BLK], BF, tag="zt")
            n1_sb = mats.tile([P, KO, BLK], BF, tag="n1")
            n2_sb = mats.tile([P, KO, BLK], BF, tag="n2")

            ag1_in = dram.tile([D, BLK], BF)
            ag2_in = dram.tile([D, BLK], BF)
            ag1_out = dram.tile([NCORES, D, BLK], BF, addr_space="Shared")
            ag2_out = dram.tile([NCORES, D, BLK], BF, addr_space="Shared")
            rs_in = dram.tile([N], F32)
            rs_out = dram.tile([BLK], F32)
            rn_dram = dram.tile([2, BLK], BF)
            p_dram = dram.tile([BLK], F32)

            # ------------ projection + normalize (into n_sb), per tensor ------------
            def project(z_at, elu_sb, n_sb, rn_slot):
                # layer 1: a1T[o, i] = W1T.T @ zT (K=d);
                # elu+1 = relu(a+b1) + min(exp(a+b1), 1)
                for ot in range(KO):
                    ps = psA.tile([P, 1024], F32, tag="ps_big")
                    for ch in range(2):
                        sl = bass.ts(ch, 512)
                        for kt in range(KO):
                            nc.tensor.matmul(
                                ps[:, sl],
                                w1_sb[:, kt, bass.ts(ot, P)],
                                z_at(kt, ch),
                                start=(kt == 0),
                                stop=(kt == KO - 1),
                            )
                    bcol = b1_sb[:, ot : ot + 1]
                    e_t = scratch.tile([P, 1024], F32, tag="e_t")
                    r_t = scratch.tile([P, 1024], F32, tag="r_t")
                    nc.scalar.activation(e_t[:], ps[:], AF.Exp, bias=bcol)
                    nc.scalar.activation(r_t[:], ps[:], AF.Relu, bias=bcol)
                    nc.vector.tensor_scalar(e_t[:], e_t[:], 1.0, None, ALU.min)
                    nc.vector.tensor_tensor(elu_sb[:, ot, :], e_t[:], r_t[:], ALU.add)
                # layer 2 -> n_sb (holds hT until scaled in place)
                for ot in range(KO):
                    ps = psA.tile([P, 1024], F32, tag="ps_big")
                    for ch in range(2):
                        sl = bass.ts(ch, 512)
                        for kt in range(KO):
                            nc.tensor.matmul(
                                ps[:, sl],
                                w2_sb[:, kt, bass.ts(ot, P)],
                                elu_sb[:, kt, bass.ds(ch * 512, 512)],
                                start=(kt == 0),
                                stop=(kt == KO - 1),
                            )
                    nc.vector.tensor_scalar(
                        n_sb[:, ot, :], ps[:], b2_sb[:, ot : ot + 1], None, ALU.add
                    )
                # sumsq over d (partitions) via ones-matmul on Square(h)
                ssps = [psB.tile([1, 512], F32, name=f"ssps{_c}", tag="ps_small") for _c in range(2)]
                for kt in range(KO):
                    sq = scratch.tile([P, BLK], BF, tag="sq")
                    nc.scalar.activation(sq[:], n_sb[:, kt, :], AF.Square)
                    for ch in range(2):
                        nc.tensor.matmul(
                            ssps[ch][:],
                            ones_bf[:],
                            sq[:, bass.ts(ch, 512)],
                            start=(kt == 0),
                            stop=(kt == KO - 1),
                        )
                # rn = 1/||h|| per column, one Newton step on top of 1/sqrt
                rn_bf = small.tile([1, BLK], BF, tag="rn_bf")
                for ch in range(2):
                    sl = bass.ts(ch, 512)
                    ssq_c = small.tile([1, 512], F32, tag="ssq_c", name=f"ssq_c{ch}")
                    nrm_c = small.tile([1, 512], F32, tag="nrm_c", name=f"nrm_c{ch}")
                    y_c = small.tile([1, 512], F32, tag="y_c", name=f"y_c{ch}")
                    t1_c = small.tile([1, 512], F32, tag="t1_c", name=f"t1_c{ch}")
                    nc.vector.tensor_copy(ssq_c[:], ssps[ch][:])
                    nc.scalar.activation(nrm_c[:], ssps[ch][:], AF.Sqrt)
                    nc.vector.reciprocal(y_c[:], nrm_c[:])
                    nc.vector.tensor_tensor(t1_c[:], y_c[:], y_c[:], ALU.mult)
                    nc.vector.tensor_tensor(t1_c[:], t1_c[:], ssq_c[:], ALU.mult)
                    nc.vector.tensor_scalar(t1_c[:], t1_c[:], -0.5, 1.5, ALU.mult, ALU.add)
                    nc.vector.tensor_tensor(t1_c[:], y_c[:], t1_c[:], ALU.mult)
                    nc.vector.tensor_copy(rn_bf[:, sl], t1_c[:])
                nc.scalar.dma_start(rn_dram[rn_slot : rn_slot + 1, :], rn_bf[:])
                rn_bc = scratch.tile([P, BLK], BF, tag="rnbc", bufs=1)
                nc.scalar.dma_start(rn_bc[:], rn_dram[rn_slot : rn_slot + 1, :].to_broadcast((P, BLK)))
                for kt in range(KO):
                    nc.vector.tensor_tensor(n_sb[:, kt, :], n_sb[:, kt, :], rn_bc[:], ALU.mult)

            rg = [list(range(NCORES))]
            # z1 into its slot; z2 into the (idle until pass A) rhs-pool slots so
            # both projections can interleave on the PE.
            nc.sync.dma_start(z_sb[:], kp(z1t[:]))
            z2a = rhsp.tile([P, KO, 512], BF, tag="rhs", name="z2a")
            z2b = rhsp.tile([P, KO, 512], BF, tag="rhs", name="z2b")
            nc.sync.dma_start(z2a[:], kp(z2t[:, 0:512]))
            nc.sync.dma_start(z2b[:], kp(z2t[:, 512:1024]))
            elu1 = mats.tile([P, KO, BLK], BF, tag="elu")
            project(lambda kt, ch: z_sb[:, kt, bass.ds(ch * 512, 512)], elu1, n1_sb, 0)
            nc.scalar.dma_start(kp(ag1_in[:]), n1_sb[:])
            nc.gpsimd.collective_compute(
                "AllGather", ALU.bypass, replica_groups=rg,
                ins=[ag1_in[:].opt()], outs=[ag1_out[:].opt()],
            )
            # elu2 reuses the z1 slot (z1 dead after its layer 1)
            elu2 = mats.tile([P, KO, BLK], BF, tag="zt", name="elu2")
            project(lambda kt, ch: (z2a if ch == 0 else z2b)[:, kt, :], elu2, n2_sb, 1)
            nc.scalar.dma_start(kp(ag2_in[:]), n2_sb[:])
            nc.gpsimd.collective_compute(
                "AllGather", ALU.bypass, replica_groups=rg,
                ins=[ag2_in[:].opt()], outs=[ag2_out[:].opt()],
            )

            # ---------------- p_i = n1_i . n2_i (local diag of S12) ----------------
            pps = [psB.tile([1, 512], F32, name=f"pps{_c}", tag="ps_small") for _c in range(2)]
            for kt in range(KO):
                q = scratch.tile([P, BLK], BF, tag="sq")
                nc.vector.tensor_tensor(q[:], n1_sb[:, kt, :], n2_sb[:, kt, :], ALU.mult)
                for ch in range(2):
                    nc.tensor.matmul(
                        pps[ch][:],
                        ones_bf[:],
                        q[:, bass.ts(ch, 512)],
                        start=(kt == 0),
                        stop=(kt == KO - 1),
                    )
            for ch in range(2):
                p_c = small.tile([1, 512], F32, tag="ssq_c", name=f"p_c{ch}")
                nc.vector.tensor_copy(p_c[:], pps[ch][:])
                nc.gpsimd.dma_start(p_dram[ch * 512 : (ch + 1) * 512], p_c[:])

            # rowsum partials, one column per j-chunk-pair
            r11p = strip.tile([P, NT, JP], F32)
            r12p = strip.tile([P, NT, JP], F32)
            r22p = strip.tile([P, NT, JP], F32)
            cs = strip.tile([P, N], F32)  # exp(2*S12) partial column sums

            def rhs_pair(ag, jp):
                a = rhsp.tile([P, KO, 512], BF, tag="rhs", name=f"rhs_a{jp}")
                b = rhsp.tile([P, KO, 512], BF, tag="rhs", name=f"rhs_b{jp}")
                blk = kp(ag[jp])
                nc.sync.dma_start(a[:], blk[:, :, 0:512])
                nc.sync.dma_start(b[:], blk[:, :, 512:1024])
                return a, b

            def sim_iter(lhs, tt, rta, rtb, accum, s12_jp=None):
                ps = psA.tile([P, 1024], F32, tag="ps_big", name="ps_sim")
                for ch, rt in ((0, rta), (1, rtb)):
                    sl = bass.ts(ch, 512)
                    for kt in range(KO):
                        nc.tensor.matmul(
                            ps[:, sl],
                            lhs[:, kt, bass.ts(tt, P)],
                            rt[:, kt, :],
                            start=(kt == 0),
                            stop=(kt == KO - 1),
                        )
                ex = expp.tile([P, 1024], F32, tag="ex")
                nc.scalar.activation(ex[:], ps[:], AF.Exp, scale=2.0, accum_out=accum)
                if s12_jp is not None:
                    csl = cs[:, bass.ds(s12_jp * 1024, 1024)]
                    nc.vector.tensor_tensor(csl, csl, ex[:], ALU.add)

            # ---- pass A: S11 (lhs n1, rhs gathered n1) ----
            for jp in range(JP):
                rta, rtb = rhs_pair(ag1_out, jp)
                for tt in range(NT):
                    sim_iter(n1_sb, tt, rta, rtb, r11p[:, tt, jp : jp + 1])

            # ---- pass B1: S12 (lhs n1, rhs gathered n2) + incremental colsums ----
            nc.vector.memset(cs[:], 0.0)
            for jp in range(JP):
                rta, rtb = rhs_pair(ag2_out, jp)
                for tt in range(NT):
                    sim_iter(n1_sb, tt, rta, rtb, r12p[:, tt, jp : jp + 1], s12_jp=jp)
                # this 1024-wide slice of cs is complete -> reduce over partitions
                for h in range(2):
                    cp = psB.tile([1, 512], F32, tag="ps_small", name=f"cp{jp}_{h}")
                    nc.tensor.matmul(
                        cp[:], ones_f[:], cs[:, bass.ds(jp * 1024 + h * 512, 512)],
                        start=True, stop=True,
                    )
                    cst = scratch.tile([1, 512], F32, tag="cst", bufs=2, name=f"cst{jp}_{h}")
                    nc.vector.tensor_copy(cst[:], cp[:])
                    nc.gpsimd.dma_start(
                        rs_in[(jp * 2 + h) * 512 : (jp * 2 + h + 1) * 512], cst[:]
                    )
            nc.gpsimd.collective_compute(
                "ReduceScatter", ALU.add, replica_groups=rg,
                ins=[rs_in[:].opt()], outs=[rs_out[:].opt()],
            )

            # ---- pass B2: S22 (lhs n2, rhs gathered n2); RS overlaps this ----
            for jp in range(JP):
                rta, rtb = rhs_pair(ag2_out, jp)
                for tt in range(NT):
                    sim_iter(n2_sb, tt, rta, rtb, r22p[:, tt, jp : jp + 1])

            # ---------------- final loss ----------------
            r11 = small.tile([P, NT], F32, tag="r11")
            r12 = small.tile([P, NT], F32, tag="r12")
            r22 = small.tile([P, NT], F32, tag="r22")
            nc.vector.reduce_sum(r11[:], r11p[:], axis=mybir.AxisListType.X)
            nc.vector.reduce_sum(r12[:], r12p[:], axis=mybir.AxisListType.X)
            nc.vector.reduce_sum(r22[:], r22p[:], axis=mybir.AxisListType.X)
            c12 = small.tile([P, NT], F32, tag="c12")
            nc.sync.dma_start(c12[:], pt(rs_out[:]))
            p2 = small.tile([P, NT], F32, tag="p2")
            nc.sync.dma_start(p2[:], pt(p_dram[:]))

            d1 = small.tile([P, NT], F32, tag="d1")
            d2 = small.tile([P, NT], F32, tag="d2")
            nc.vector.tensor_tensor(d1[:], r11[:], r12[:], ALU.add)
            nc.vector.tensor_scalar(d1[:], d1[:], -E2, None, ALU.add)
            nc.vector.tensor_tensor(d2[:], r22[:], c12[:], ALU.add)
            nc.vector.tensor_scalar(d2[:], d2[:], -E2, None, ALU.add)
            l1 = small.tile([P, NT], F32, tag="l1")
            l2 = small.tile([P, NT], F32, tag="l2")
            nc.scalar.activation(l1[:], d1[:], AF.Ln)
            nc.scalar.activation(l2[:], d2[:], AF.Ln)
            loss = small.tile([P, NT], F32, tag="loss")
            nc.vector.tensor_tensor(loss[:], l1[:], l2[:], ALU.add)
            nc.vector.tensor_scalar(loss[:], loss[:], 0.5, None, ALU.mult)
            pm = small.tile([P, NT], F32, tag="pm")
            nc.vector.tensor_scalar(pm[:], p2[:], -2.0, None, ALU.mult)
            nc.vector.tensor_tensor(loss[:], loss[:], pm[:], ALU.add)
            nc.sync.dma_start(pt(out[:]), loss[:])

    nc.finalize()
    return nc


@lru_cache(maxsize=1)
def _built():
    return _build()


def _prep_inputs(z1, z2, fc1_w, fc1_b, fc2_w, fc2_b):
    bf = ml_dtypes.bfloat16
    w1t = np.ascontiguousarray(np.asarray(fc1_w, np.float32).T).astype(bf)
    w2t = np.ascontiguousarray(np.asarray(fc2_w, np.float32).T).astype(bf)
    b1 = np.asarray(fc1_b, np.float32)
    b2p = (np.asarray(fc2_b, np.float32) - np.asarray(fc2_w, np.float32).sum(axis=1)).astype(
        np.float32
    )
    in_maps = []
    for c in range(NCORES):
        sl = slice(c * BLK, (c + 1) * BLK)
        in_maps.append(
            {
                "z1t": np.ascontiguousarray(np.asarray(z1[sl], np.float32).T).astype(bf),
                "z2t": np.ascontiguousarray(np.asarray(z2[sl], np.float32).T).astype(bf),
                "w1t": w1t,
                "w2t": w2t,
                "b1": b1,
                "b2p": b2p,
            }
        )
    return in_maps


def _install_ntff_shim():
    """Register the axon NTFF profile hook (antenv.axon_hooks is absent in
    this image; rebuild it from trn_agent_boot's ctypes recipe)."""
    import sys
    import types

    if "antenv.axon_hooks" in sys.modules:
        return True
    try:
        import antenv
        from trn_agent_boot.trn_boot import _ntff_profile_via_ctypes

        hook = _ntff_profile_via_ctypes("/opt/axon/libaxon_pjrt.so")
        if hook is None:
            return False
        m = types.ModuleType("antenv.axon_hooks")
        m._hook = hook
        m.get_axon_ntff_profile_hook = lambda: m._hook
        m.set_axon_ntff_profile_hook = lambda h: setattr(m, "_hook", h)
        sys.modules["antenv.axon_hooks"] = m
        antenv.axon_hooks = m
        # artifact upload needs egress; neuter it for local profiling
        import concourse.bass_utils as _bu

        _bu.upload_artifacts = lambda tmpdir: f"file://{tmpdir}"
        return True
    except Exception as e:
        print(f"ntff shim unavailable: {e!r}")
        return False


def _run(in_maps, trace=False):
    nc = _built()
    if trace and not _install_ntff_shim():
        trace = False
    last = None
    for attempt in range(3):
        try:
            res = run_bass_kernel_spmd(nc, in_maps, list(range(NCORES)), trace=trace)
            if all(np.isfinite(res.results[c]["out"]).all() for c in range(NCORES)):
                return res
            print("nonfinite output, retrying")
        except Exception as e:  # device occasionally wedged from a prior process
            last = e
            if "UNRECOVERABLE" not in str(e) and "UNAVAILABLE" not in str(e):
                raise
            print(f"device error (attempt {attempt}): retrying")
    if last is not None:
        raise last
    return res


def kernel(z1, z2, fc1_w, fc1_b, fc2_w, fc2_b):
    in_maps = _prep_inputs(z1, z2, fc1_w, fc1_b, fc2_w, fc2_b)
    res = _run(in_maps, trace=os.environ.get("KERNEL_TRACE", "") == "1")
    if res.exec_time_ns is not None:
        print(f"HW exec time: {res.exec_time_ns} ns")
    out = np.concatenate([res.results[c]["out"] for c in range(NCORES)])
    return out.astype(np.float32)



# revision 11
# speedup vs baseline: 1.6150x; 1.6150x over previous
"""Trainium2 Bass kernel for nn_CLLayer (SimCLR-style contrastive loss).

Math (reference, tau=0.5):
    h1 = elu(z1 @ W1.T + b1) @ W2.T + b2 ; h2 likewise
    n1, n2 = row-normalized h1, h2
    l1_i = log(sum_j exp(2*n1_i.n1_j) + sum_j exp(2*n1_i.n2_j) - e^2) - 2*n1_i.n2_i
    l2_i = log(sum_j exp(2*n2_i.n2_j) + sum_j exp(2*n2_j.n1_i... ) - e^2) - 2*...
    out = 0.5*(l1+l2)

Sharding: row-parallel over N=8192 (1024 rows/core, 8 cores).
Each core: projects its row block (bf16 matmuls), normalizes, AllGathers
normalized embeddings (bf16), computes its row-strip of the three distinct
similarity products (S12, S22, S11), exp+row-sums on the fly, column-sums of
exp(2*S12) via a ReduceScatter (between2 = between.T so l2's "between" row
sums are column sums of S12's exp).  Only 3 of 4 N^2*D products are needed.

Host-side prep: transposes z blocks / weights to K-major (PE wants K on
partitions), casts matmul operands to bf16, and folds the ELU "-1" into an
adjusted fc2 bias (b2' = b2 - fc2_w.sum(1)) so ELU is computed as
relu(x) + exp(min(x,0)) without the subtract (device ELU' = elu + 1).
"""

import math
import os
from functools import lru_cache

import ml_dtypes
import numpy as np

import concourse.bacc as bacc
import concourse.bass as bass
import concourse.mybir as mybir
import concourse.tile as tile
from concourse.bass_utils import run_bass_kernel_spmd

N, D = 8192, 1024
NCORES = 8
BLK = N // NCORES  # 1024
P = 128
KO = D // P  # 8 k-tiles
NT = BLK // P  # 8 i-tiles per core
JC = N // 512  # 16 j-chunks of 512
E2 = float(np.exp(2.0))  # exp(1/tau), tau=0.5
BF = mybir.dt.bfloat16
F32 = mybir.dt.float32
F8 = mybir.dt.float8e4
AF = mybir.ActivationFunctionType
ALU = mybir.AluOpType
PM2 = mybir.MatmulPerfMode.DoubleRow
QS = 16.0  # fp8 quant scale for normalized embeddings (entries ~N(0, 1/4))
IQS2 = 1.0 / (QS * QS)  # sim psum holds 256*S


def _build():
    nc = bacc.Bacc("TRN2", target_bir_lowering=False, debug=False, num_devices=NCORES)

    z1t = nc.dram_tensor("z1t", [D, BLK], BF, kind="ExternalInput")
    z2t = nc.dram_tensor("z2t", [D, BLK], BF, kind="ExternalInput")
    w1t = nc.dram_tensor("w1t", [D, D], BF, kind="ExternalInput")
    w2t = nc.dram_tensor("w2t", [D, D], BF, kind="ExternalInput")
    b1 = nc.dram_tensor("b1", [D], F32, kind="ExternalInput")
    b2p = nc.dram_tensor("b2p", [D], F32, kind="ExternalInput")
    out = nc.dram_tensor("out", [BLK], F32, kind="ExternalOutput")

    kp = lambda ap: ap.rearrange("(ko ki) x -> ki ko x", ki=P)  # K-major -> [128, KO, x]
    pt = lambda ap: ap.rearrange("(t p) -> p t", p=P)  # [1024] -> [128, 8]
    JP = JC // 2  # 8 j-chunk-pairs of 1024

    with tile.TileContext(nc) as tc:
        with (
            tc.tile_pool(name="consts", bufs=1) as consts,
            tc.tile_pool(name="mats", bufs=1) as mats,
            tc.tile_pool(name="strip", bufs=1) as strip,
            tc.tile_pool(name="scratch", bufs=2) as scratch,
            tc.tile_pool(name="rhs", bufs=3) as rhsp,
            tc.tile_pool(name="expp", bufs=2) as expp,
            tc.tile_pool(name="small", bufs=1) as small,
            tc.tile_pool(name="psA", bufs=3, space="PSUM") as psA,
            tc.tile_pool(name="psB", bufs=2, space="PSUM") as psB,
            tc.tile_pool(name="dram", bufs=1, space="DRAM") as dram,
        ):
            # ---------------- constants ----------------
            w1_sb = consts.tile([P, KO, D], BF)
            w2_sb = consts.tile([P, KO, D], BF)
            nc.sync.dma_start(w1_sb[:], kp(w1t[:]))
            nc.sync.dma_start(w2_sb[:], kp(w2t[:]))
            b1_sb = consts.tile([P, KO], F32)
            b2_sb = consts.tile([P, KO], F32)
            nc.sync.dma_start(b1_sb[:], pt(b1[:]))
            nc.sync.dma_start(b2_sb[:], pt(b2p[:]))
            ones_bf = consts.tile([P, 1], BF)
            ones_f = consts.tile([P, 1], F32)
            nc.vector.memset(ones_bf[:], 1.0)
            nc.vector.memset(ones_f[:], 1.0)

            z_sb = mats.tile([P, KO, BLK], BF, tag="zt")
            n1_sb = mats.tile([P, KO, BLK], BF, tag="n1")
            n2_sb = mats.tile([P, KO, BLK], BF, tag="n2")
            n1q = mats.tile([P, KO, BLK], F8, tag="n1q")
            n2q = mats.tile([P, KO, BLK], F8, tag="n2q")

            ag1_in = dram.tile([D, BLK], F8)
            ag2_in = dram.tile([D, BLK], F8)
            ag1_out = dram.tile([NCORES, D, BLK], F8, addr_space="Shared")
            ag2_out = dram.tile([NCORES, D, BLK], F8, addr_space="Shared")
            rs_in = dram.tile([N], F32)
            rs_out = dram.tile([BLK], F32)
            rn_dram = dram.tile([2, BLK], BF)
            p_dram = dram.tile([BLK], F32)

            # ------- projection + normalize (h into n_sb, fp8 QS*n into nq_sb) -------
            def project(z_at, elu_sb, n_sb, nq_sb, rn_slot):
                # layer 1: a1T[o, i] = W1T.T @ zT (K=d);
                # elu+1 = relu(a+b1) + min(exp(a+b1), 1)
                for ot in range(KO):
                    ps = psA.tile([P, 1024], F32, tag="ps_big")
                    for ch in range(2):
                        sl = bass.ts(ch, 512)
                        for kt in range(KO):
                            nc.tensor.matmul(
                                ps[:, sl],
                                w1_sb[:, kt, bass.ts(ot, P)],
                                z_at(kt, ch),
                                start=(kt == 0),
                                stop=(kt == KO - 1),
                            )
                    bcol = b1_sb[:, ot : ot + 1]
                    e_t = scratch.tile([P, 1024], F32, tag="e_t")
                    r_t = scratch.tile([P, 1024], F32, tag="r_t")
                    nc.scalar.activation(e_t[:], ps[:], AF.Exp, bias=bcol)
                    nc.scalar.activation(r_t[:], ps[:], AF.Relu, bias=bcol)
                    nc.vector.tensor_scalar(e_t[:], e_t[:], 1.0, None, ALU.min)
                    nc.vector.tensor_tensor(elu_sb[:, ot, :], e_t[:], r_t[:], ALU.add)
                # layer 2 -> n_sb (holds hT until scaled in place)
                for ot in range(KO):
                    ps = psA.tile([P, 1024], F32, tag="ps_big")
                    for ch in range(2):
                        sl = bass.ts(ch, 512)
                        for kt in range(KO):
                            nc.tensor.matmul(
                                ps[:, sl],
                                w2_sb[:, kt, bass.ts(ot, P)],
                                elu_sb[:, kt, bass.ds(ch * 512, 512)],
                                start=(kt == 0),
                                stop=(kt == KO - 1),
                            )
                    nc.vector.tensor_scalar(
                        n_sb[:, ot, :], ps[:], b2_sb[:, ot : ot + 1], None, ALU.add
                    )
                # sumsq over d (partitions) via ones-matmul on Square(h)
                ssps = [psB.tile([1, 512], F32, name=f"ssps{_c}", tag="ps_small") for _c in range(2)]
                for kt in range(KO):
                    sq = scratch.tile([P, BLK], BF, tag="sq")
                    nc.scalar.activation(sq[:], n_sb[:, kt, :], AF.Square)
                    for ch in range(2):
                        nc.tensor.matmul(
                            ssps[ch][:],
                            ones_bf[:],
                            sq[:, bass.ts(ch, 512)],
                            start=(kt == 0),
                            stop=(kt == KO - 1),
                        )
                # rn = QS/||h|| per column, one Newton step on top of 1/sqrt
                rn_bf = small.tile([1, BLK], BF, tag="rn_bf")
                for ch in range(2):
                    sl = bass.ts(ch, 512)
                    ssq_c = small.tile([1, 512], F32, tag="ssq_c", name=f"ssq_c{ch}")
                    nrm_c = small.tile([1, 512], F32, tag="nrm_c", name=f"nrm_c{ch}")
                    y_c = small.tile([1, 512], F32, tag="y_c", name=f"y_c{ch}")
                    t1_c = small.tile([1, 512], F32, tag="t1_c", name=f"t1_c{ch}")
                    nc.vector.tensor_copy(ssq_c[:], ssps[ch][:])
                    nc.scalar.activation(nrm_c[:], ssps[ch][:], AF.Sqrt)
                    nc.vector.reciprocal(y_c[:], nrm_c[:])
                    nc.vector.tensor_tensor(t1_c[:], y_c[:], y_c[:], ALU.mult)
                    nc.vector.tensor_tensor(t1_c[:], t1_c[:], ssq_c[:], ALU.mult)
                    nc.vector.tensor_scalar(t1_c[:], t1_c[:], -0.5, 1.5, ALU.mult, ALU.add)
                    nc.vector.tensor_tensor(t1_c[:], y_c[:], t1_c[:], ALU.mult)
                    nc.vector.tensor_scalar(rn_bf[:, sl], t1_c[:], QS, None, ALU.mult)
                nc.scalar.dma_start(rn_dram[rn_slot : rn_slot + 1, :], rn_bf[:])
                rn_bc = scratch.tile([P, BLK], BF, tag="rnbc", bufs=1)
                nc.scalar.dma_start(rn_bc[:], rn_dram[rn_slot : rn_slot + 1, :].to_broadcast((P, BLK)))
                for kt in range(KO):
                    nc.vector.tensor_tensor(nq_sb[:, kt, :], n_sb[:, kt, :], rn_bc[:], ALU.mult)

            rg = [list(range(NCORES))]
            # z1 into its slot; z2 into the (idle until pass A) rhs-pool slots so
            # both projections can interleave on the PE.
            nc.sync.dma_start(z_sb[:], kp(z1t[:]))
            z2a = rhsp.tile([P, KO, 512], BF, tag="rhs", name="z2a")
            z2b = rhsp.tile([P, KO, 512], BF, tag="rhs", name="z2b")
            nc.sync.dma_start(z2a[:], kp(z2t[:, 0:512]))
            nc.sync.dma_start(z2b[:], kp(z2t[:, 512:1024]))
            elu1 = mats.tile([P, KO, BLK], BF, tag="elu")
            project(lambda kt, ch: z_sb[:, kt, bass.ds(ch * 512, 512)], elu1, n1_sb, n1q, 0)
            nc.scalar.dma_start(kp(ag1_in[:]), n1q[:])
            nc.gpsimd.collective_compute(
                "AllGather", ALU.bypass, replica_groups=rg,
                ins=[ag1_in[:].opt()], outs=[ag1_out[:].opt()],
            )
            # elu2 reuses the z1 slot (z1 dead after its layer 1)
            elu2 = mats.tile([P, KO, BLK], BF, tag="zt", name="elu2")
            project(lambda kt, ch: (z2a if ch == 0 else z2b)[:, kt, :], elu2, n2_sb, n2q, 1)
            nc.scalar.dma_start(kp(ag2_in[:]), n2q[:])
            nc.gpsimd.collective_compute(
                "AllGather", ALU.bypass, replica_groups=rg,
                ins=[ag2_in[:].opt()], outs=[ag2_out[:].opt()],
            )

            # ------- p_i = n1_i . n2_i (local diag of S12; psum holds 256*p) -------
            pps = [psB.tile([1, 512], F32, name=f"pps{_c}", tag="ps_small") for _c in range(2)]
            for kt in range(KO):
                q = scratch.tile([P, BLK], BF, tag="sq")
                nc.vector.tensor_tensor(q[:], n1q[:, kt, :], n2q[:, kt, :], ALU.mult)
                for ch in range(2):
                    nc.tensor.matmul(
                        pps[ch][:],
                        ones_bf[:],
                        q[:, bass.ts(ch, 512)],
                        start=(kt == 0),
                        stop=(kt == KO - 1),
                    )
            for ch in range(2):
                p_c = small.tile([1, 512], F32, tag="ssq_c", name=f"p_c{ch}")
                nc.vector.tensor_copy(p_c[:], pps[ch][:])
                nc.gpsimd.dma_start(p_dram[ch * 512 : (ch + 1) * 512], p_c[:])

            # rowsum partials, one column per j-chunk-pair
            r11p = strip.tile([P, NT, JP], F32)
            r12p = strip.tile([P, NT, JP], F32)
            r22p = strip.tile([P, NT, JP], F32)

            def rhs_pair(ag, jp):
                a = rhsp.tile([P, KO, 512], F8, tag="rhs", name=f"rhs_a{jp}")
                b = rhsp.tile([P, KO, 512], F8, tag="rhs", name=f"rhs_b{jp}")
                blk = kp(ag[jp])
                nc.sync.dma_start(a[:], blk[:, :, 0:512])
                nc.sync.dma_start(b[:], blk[:, :, 512:1024])
                return a, b

            def sim_iter(lhs, tt, rta, rtb, accum, cs_t=None):
                ps = psA.tile([P, 1024], F32, tag="ps_big", name="ps_sim")
                for ch, rt in ((0, rta), (1, rtb)):
                    sl = bass.ts(ch, 512)
                    for kt in range(0, KO, 2):
                        nc.tensor.matmul(
                            ps[:, sl],
                            lhs[:, kt : kt + 2, bass.ts(tt, P)],
                            rt[:, kt : kt + 2, :],
                            start=(kt == 0),
                            stop=(kt == KO - 2),
                            perf_mode=PM2,
                        )
                ex = expp.tile([P, 1024], F32, tag="ex")
                nc.scalar.activation(ex[:], ps[:], AF.Exp, scale=2.0 * IQS2, accum_out=accum)
                if cs_t is not None:
                    if tt == 0:
                        nc.vector.tensor_copy(cs_t[:], ex[:])
                    else:
                        nc.vector.tensor_tensor(cs_t[:], cs_t[:], ex[:], ALU.add)

            # ---- pass A: S11 (lhs n1, rhs gathered n1) ----
            for jp in range(JP):
                rta, rtb = rhs_pair(ag1_out, jp)
                for tt in range(NT):
                    sim_iter(n1q, tt, rta, rtb, r11p[:, tt, jp : jp + 1])

            # ---- pass B1: S12 (lhs n1, rhs gathered n2) + incremental colsums ----
            for jp in range(JP):
                rta, rtb = rhs_pair(ag2_out, jp)
                cs_t = scratch.tile([P, 1024], F32, tag="cs", name=f"cs{jp}")
                for tt in range(NT):
                    sim_iter(n1q, tt, rta, rtb, r12p[:, tt, jp : jp + 1], cs_t=cs_t)
                # this 1024-wide column-sum slice is complete -> reduce over partitions
                for h in range(2):
                    cp = psB.tile([1, 512], F32, tag="ps_small", name=f"cp{jp}_{h}")
                    nc.tensor.matmul(
                        cp[:], ones_f[:], cs_t[:, bass.ts(h, 512)],
                        start=True, stop=True,
                    )
                    cst = scratch.tile([1, 512], F32, tag="cst", bufs=2, name=f"cst{jp}_{h}")
                    nc.vector.tensor_copy(cst[:], cp[:])
                    nc.gpsimd.dma_start(
                        rs_in[(jp * 2 + h) * 512 : (jp * 2 + h + 1) * 512], cst[:]
                    )
            nc.gpsimd.collective_compute(
                "ReduceScatter", ALU.add, replica_groups=rg,
                ins=[rs_in[:].opt()], outs=[rs_out[:].opt()],
            )

            # ---- pass B2: S22 (lhs n2, rhs gathered n2); RS overlaps this ----
            for jp in range(JP):
                rta, rtb = rhs_pair(ag2_out, jp)
                for tt in range(NT):
                    sim_iter(n2q, tt, rta, rtb, r22p[:, tt, jp : jp + 1])

            # ---------------- final loss ----------------
            r11 = small.tile([P, NT], F32, tag="r11")
            r12 = small.tile([P, NT], F32, tag="r12")
            r22 = small.tile([P, NT], F32, tag="r22")
            nc.vector.reduce_sum(r11[:], r11p[:], axis=mybir.AxisListType.X)
            nc.vector.reduce_sum(r12[:], r12p[:], axis=mybir.AxisListType.X)
            nc.vector.reduce_sum(r22[:], r22p[:], axis=mybir.AxisListType.X)
            c12 = small.tile([P, NT], F32, tag="c12")
            nc.sync.dma_start(c12[:], pt(rs_out[:]))
            p2 = small.tile([P, NT], F32, tag="p2")
            nc.sync.dma_start(p2[:], pt(p_dram[:]))

            d1 = small.tile([P, NT], F32, tag="d1")
            d2 = small.tile([P, NT], F32, tag="d2")
            nc.vector.tensor_tensor(d1[:], r11[:], r12[:], ALU.add)
            nc.vector.tensor_scalar(d1[:], d1[:], -E2, None, ALU.add)
            nc.vector.tensor_tensor(d2[:], r22[:], c12[:], ALU.add)
            nc.vector.tensor_scalar(d2[:], d2[:], -E2, None, ALU.add)
            l1 = small.tile([P, NT], F32, tag="l1")
            l2 = small.tile([P, NT], F32, tag="l2")
            nc.scalar.activation(l1[:], d1[:], AF.Ln)
            nc.scalar.activation(l2[:], d2[:], AF.Ln)
            loss = small.tile([P, NT], F32, tag="loss")
            nc.vector.tensor_tensor(loss[:], l1[:], l2[:], ALU.add)
            nc.vector.tensor_scalar(loss[:], loss[:], 0.5, None, ALU.mult)
            pm = small.tile([P, NT], F32, tag="pm")
            nc.vector.tensor_scalar(pm[:], p2[:], -2.0 * IQS2, None, ALU.mult)
            nc.vector.tensor_tensor(loss[:], loss[:], pm[:], ALU.add)
            nc.sync.dma_start(pt(out[:]), loss[:])

    nc.finalize()
    return nc


@lru_cache(maxsize=1)
def _built():
    return _build()


def _prep_inputs(z1, z2, fc1_w, fc1_b, fc2_w, fc2_b):
    bf = ml_dtypes.bfloat16
    w1t = np.ascontiguousarray(np.asarray(fc1_w, np.float32).T).astype(bf)
    w2t = np.ascontiguousarray(np.asarray(fc2_w, np.float32).T).astype(bf)
    b1 = np.asarray(fc1_b, np.float32)
    b2p = (np.asarray(fc2_b, np.float32) - np.asarray(fc2_w, np.float32).sum(axis=1)).astype(
        np.float32
    )
    in_maps = []
    for c in range(NCORES):
        sl = slice(c * BLK, (c + 1) * BLK)
        in_maps.append(
            {
                "z1t": np.ascontiguousarray(np.asarray(z1[sl], np.float32).T).astype(bf),
                "z2t": np.ascontiguousarray(np.asarray(z2[sl], np.float32).T).astype(bf),
                "w1t": w1t,
                "w2t": w2t,
                "b1": b1,
                "b2p": b2p,
            }
        )
    return in_maps


def _install_ntff_shim():
    """Register the axon NTFF profile hook (antenv.axon_hooks is absent in
    this image; rebuild it from trn_agent_boot's ctypes recipe)."""
    import sys
    import types

    if "antenv.axon_hooks" in sys.modules:
        return True
    try:
        import antenv
        from trn_agent_boot.trn_boot import _ntff_profile_via_ctypes

        hook = _ntff_profile_via_ctypes("/opt/axon/libaxon_pjrt.so")
        if hook is None:
            return False
        m = types.ModuleType("antenv.axon_hooks")
        m._hook = hook
        m.get_axon_ntff_profile_hook = lambda: m._hook
        m.set_axon_ntff_profile_hook = lambda h: setattr(m, "_hook", h)
        sys.modules["antenv.axon_hooks"] = m
        antenv.axon_hooks = m
        # artifact upload needs egress; neuter it for local profiling
        import concourse.bass_utils as _bu

        _bu.upload_artifacts = lambda tmpdir: f"file://{tmpdir}"
        return True
    except Exception as e:
        print(f"ntff shim unavailable: {e!r}")
        return False


def _run(in_maps, trace=False):
    nc = _built()
    if trace and not _install_ntff_shim():
        trace = False
    last = None
    for attempt in range(3):
        try:
            res = run_bass_kernel_spmd(nc, in_maps, list(range(NCORES)), trace=trace)
            if all(np.isfinite(res.results[c]["out"]).all() for c in range(NCORES)):
                return res
            print("nonfinite output, retrying")
        except Exception as e:  # device occasionally wedged from a prior process
            last = e
            if "UNRECOVERABLE" not in str(e) and "UNAVAILABLE" not in str(e):
                raise
            print(f"device error (attempt {attempt}): retrying")
    if last is not None:
        raise last
    return res


def kernel(z1, z2, fc1_w, fc1_b, fc2_w, fc2_b):
    in_maps = _prep_inputs(z1, z2, fc1_w, fc1_b, fc2_w, fc2_b)
    res = _run(in_maps, trace=os.environ.get("KERNEL_TRACE", "") == "1")
    if res.exec_time_ns is not None:
        print(f"HW exec time: {res.exec_time_ns} ns")
    out = np.concatenate([res.results[c]["out"] for c in range(NCORES)])
    return out.astype(np.float32)



# revision 14
# speedup vs baseline: 2.1737x; 1.3459x over previous
"""Trainium2 Bass kernel for nn_CLLayer (SimCLR-style contrastive loss).

Math (reference, tau=0.5):
    h1 = elu(z1 @ W1.T + b1) @ W2.T + b2 ; h2 likewise
    n1, n2 = row-normalized h1, h2
    l1_i = log(sum_j exp(2*n1_i.n1_j) + sum_j exp(2*n1_i.n2_j) - e^2) - 2*n1_i.n2_i
    l2_i = log(sum_j exp(2*n2_i.n2_j) + colsum_i(exp(2*S12)) - e^2) - 2*n1_i.n2_i
    out = 0.5*(l1+l2)

Sharding: row-parallel over N=8192 (1024 rows/core, 8 cores).

All matmuls run in fp8e4 DoubleRow mode (2x PE rate): host quantizes
W (x16) and z to fp8; device quantizes normalized embeddings (x16) to
fp8 and AllGathers them (1MB/core per tensor).

Work split per core (identical on every core; rank enters only through
partition_id-driven dynamic DMA offsets):
  - S12 strip: all 8 column blocks (no symmetry).
  - S11: diag block + rotated offsets o=1..3 (colsums shared to row
    owners via ReduceScatter) + offset 4 computed rowsum-only on both
    ends of the pair.  S22 mirrored (offsets 5..7 shared, 4 unshared).
  - 18 block-products/core instead of 24; diag blocks need no gathered
    data so they run while the AllGathers are in flight.
Three colsum-share vectors ride one fused [3,N] ReduceScatter.

Host-side prep: K-major transposes, fp8 casts (weights x16, descaled on
device via activation scale=1/16), and the ELU "-1" folded into an
adjusted fc2 bias computed from the *quantized* W2 so the fold is exact.
"""

import math
import os
from functools import lru_cache

import ml_dtypes
import numpy as np

import concourse.bacc as bacc
import concourse.bass as bass
import concourse.mybir as mybir
import concourse.tile as tile
from concourse.bass_utils import run_bass_kernel_spmd

N, D = 8192, 1024
NCORES = 8
BLK = N // NCORES  # 1024
P = 128
KO = D // P  # 8 k-tiles
NT = BLK // P  # 8 i-tiles per core
E2 = float(np.exp(2.0))  # exp(1/tau), tau=0.5
BF = mybir.dt.bfloat16
F32 = mybir.dt.float32
F8 = mybir.dt.float8e4
AF = mybir.ActivationFunctionType
ALU = mybir.AluOpType
PM2 = mybir.MatmulPerfMode.DoubleRow
WS = 16.0  # host fp8 weight scale (descaled via activation scale)
IWS = 1.0 / WS
QS = 16.0  # fp8 quant scale for normalized embeddings (entries ~N(0, 1/4))
IQS2 = 1.0 / (QS * QS)  # sim psum holds 256*S


def _build():
    nc = bacc.Bacc("TRN2", target_bir_lowering=False, debug=False, num_devices=NCORES)

    z1t = nc.dram_tensor("z1t", [D, BLK], F8, kind="ExternalInput")
    z2t = nc.dram_tensor("z2t", [D, BLK], F8, kind="ExternalInput")
    w1t = nc.dram_tensor("w1t", [D, D], F8, kind="ExternalInput")
    w2t = nc.dram_tensor("w2t", [D, D], F8, kind="ExternalInput")
    b1 = nc.dram_tensor("b1", [D], F32, kind="ExternalInput")
    b2p = nc.dram_tensor("b2p", [D], F32, kind="ExternalInput")
    out = nc.dram_tensor("out", [BLK], F32, kind="ExternalOutput")

    kp = lambda ap: ap.rearrange("(ko ki) x -> ki ko x", ki=P)  # K-major -> [128, KO, x]
    pt = lambda ap: ap.rearrange("(t p) -> p t", p=P)  # [1024] -> [128, 8]

    with tile.TileContext(nc) as tc:
        with (
            tc.tile_pool(name="consts", bufs=1) as consts,
            tc.tile_pool(name="mats", bufs=1) as mats,
            tc.tile_pool(name="strip", bufs=1) as strip,
            tc.tile_pool(name="scratch", bufs=2) as scratch,
            tc.tile_pool(name="rhs", bufs=3) as rhsp,
            tc.tile_pool(name="expp", bufs=2) as expp,
            tc.tile_pool(name="small", bufs=1) as small,
            tc.tile_pool(name="psA", bufs=3, space="PSUM") as psA,
            tc.tile_pool(name="psB", bufs=2, space="PSUM") as psB,
            tc.tile_pool(name="dram", bufs=1, space="DRAM") as dram,
        ):
            pid_s = nc.sync.partition_id()
            pid_g = nc.gpsimd.partition_id()

            # ---------------- constants (z1/w1 first: they gate matmul #1) ----------
            w1_sb = consts.tile([P, KO, D], F8)
            w2_sb = consts.tile([P, KO, D], F8)
            z_sb = mats.tile([P, KO, BLK], F8, tag="zt")
            nc.sync.dma_start(z_sb[:], kp(z1t[:]))
            for ot in range(KO):
                nc.sync.dma_start(
                    w1_sb[:, :, bass.ts(ot, P)], kp(w1t[:])[:, :, bass.ts(ot, P)]
                )
            b1_sb = consts.tile([P, KO], F32)
            b2_sb = consts.tile([P, KO], F32)
            nc.sync.dma_start(b1_sb[:], pt(b1[:]))
            nc.sync.dma_start(b2_sb[:], pt(b2p[:]))
            ones_bf = consts.tile([P, 1], BF)
            nc.vector.memset(ones_bf[:], 1.0)
            for ot in range(KO):
                nc.sync.dma_start(
                    w2_sb[:, :, bass.ts(ot, P)], kp(w2t[:])[:, :, bass.ts(ot, P)]
                )
            z2a = rhsp.tile([P, KO, 512], F8, tag="rhs", name="z2a")
            z2b = rhsp.tile([P, KO, 512], F8, tag="rhs", name="z2b")
            nc.sync.dma_start(z2a[:], kp(z2t[:, 0:512]))
            nc.sync.dma_start(z2b[:], kp(z2t[:, 512:1024]))

            h1 = [mats.tile([P, BLK], BF, tag=f"h1_{j}", name=f"h1_{j}") for j in range(KO)]
            h2 = [mats.tile([P, BLK], BF, tag=f"h2_{j}", name=f"h2_{j}") for j in range(KO)]
            elu1 = mats.tile([P, KO, BLK], F8, tag="elu")
            n1q = mats.tile([P, KO, BLK], F8, tag="n1q")
            n2q = mats.tile([P, KO, BLK], F8, tag="n2q")

            ag1_in = dram.tile([D, BLK], F8)
            ag2_in = dram.tile([D, BLK], F8)
            ag1_out = dram.tile([NCORES, D, BLK], F8, addr_space="Shared")
            ag2_out = dram.tile([NCORES, D, BLK], F8, addr_space="Shared")
            # rs rows: 0 = S12 colsums, 1 = S11 colsum shares, 2 = S22 colsum shares
            # (leading dim = destination core: RS scatters contiguous chunks)
            rs_in = dram.tile([NCORES, 3, BLK], F32)
            rs_out = dram.tile([3, BLK], F32)
            rn_dram = dram.tile([2, BLK], BF)
            p_dram = dram.tile([BLK], F32)

            # zero the rs slots no share will write (rank-rotated complements)
            zrow = small.tile([1, BLK], F32, tag="zrow")
            nc.vector.memset(zrow[:], 0.0)
            for o in (0, 4, 5, 6, 7):
                nc.gpsimd.dma_start(rs_in[bass.ds((pid_g + o) % 8, 1), 1, :], zrow[:])
            for o in (0, 1, 2, 3, 4):
                nc.gpsimd.dma_start(rs_in[bass.ds((pid_g + o) % 8, 1), 2, :], zrow[:])

            # ------- projection + normalize (h per-ot bf16, fp8 QS*n into nq_sb) -----
            def project(z_at, elu_sb, h_ot, nq_sb, rn_slot):
                # layer 1: a1T[o, i] = W1T.T @ zT (K=d);
                # elu+1 = relu(a+b1) + min(exp(a+b1), 1); psum holds 16*a
                for ot in range(KO):
                    ps = psA.tile([P, 1024], F32, tag="ps_big")
                    for ch in range(2):
                        sl = bass.ts(ch, 512)
                        for kt in range(0, KO, 2):
                            nc.tensor.matmul(
                                ps[:, sl],
                                w1_sb[:, kt : kt + 2, bass.ts(ot, P)],
                                z_at(kt, ch),
                                start=(kt == 0),
                                stop=(kt == KO - 2),
                                perf_mode=PM2,
                            )
                    bcol = b1_sb[:, ot : ot + 1]
                    e_t = scratch.tile([P, 1024], F32, tag="e_t")
                    r_t = scratch.tile([P, 1024], F32, tag="r_t")
                    nc.scalar.activation(e_t[:], ps[:], AF.Exp, bias=bcol, scale=IWS)
                    nc.scalar.activation(r_t[:], ps[:], AF.Relu, bias=bcol, scale=IWS)
                    nc.vector.tensor_scalar(e_t[:], e_t[:], 1.0, None, ALU.min)
                    nc.vector.tensor_tensor(elu_sb[:, ot, :], e_t[:], r_t[:], ALU.add)
                # layer 2 -> h_ot (bf16), squares fired per-ot so sumsq pipelines
                sq_ot = []
                for ot in range(KO):
                    ps = psA.tile([P, 1024], F32, tag="ps_big")
                    for ch in range(2):
                        sl = bass.ts(ch, 512)
                        for kt in range(0, KO, 2):
                            nc.tensor.matmul(
                                ps[:, sl],
                                w2_sb[:, kt : kt + 2, bass.ts(ot, P)],
                                elu_sb[:, kt : kt + 2, bass.ds(ch * 512, 512)],
                                start=(kt == 0),
                                stop=(kt == KO - 2),
                                perf_mode=PM2,
                            )
                    nc.vector.tensor_scalar(
                        h_ot[ot][:], ps[:], IWS, b2_sb[:, ot : ot + 1], ALU.mult, ALU.add
                    )
                    sq = scratch.tile([P, BLK], BF, tag=f"sq{ot}", bufs=1)
                    nc.scalar.activation(sq[:], h_ot[ot][:], AF.Square)
                    sq_ot.append(sq)
                # sumsq over d (partitions) via ones-matmul
                ssps = [
                    psB.tile([1, 512], F32, name=f"ssps{rn_slot}_{c}", tag="ps_small")
                    for c in range(2)
                ]
                for kt in range(KO):
                    for ch in range(2):
                        nc.tensor.matmul(
                            ssps[ch][:],
                            ones_bf[:],
                            sq_ot[kt][:, bass.ts(ch, 512)],
                            start=(kt == 0),
                            stop=(kt == KO - 1),
                        )
                # rn = QS/||h|| per column, one Newton step on top of 1/sqrt
                rn_bf = small.tile([1, BLK], BF, tag="rn_bf")
                for ch in range(2):
                    sl = bass.ts(ch, 512)
                    ssq_c = small.tile([1, 512], F32, tag="ssq_c", name=f"ssq_c{ch}")
                    nrm_c = small.tile([1, 512], F32, tag="nrm_c", name=f"nrm_c{ch}")
                    y_c = small.tile([1, 512], F32, tag="y_c", name=f"y_c{ch}")
                    t1_c = small.tile([1, 512], F32, tag="t1_c", name=f"t1_c{ch}")
                    nc.vector.tensor_copy(ssq_c[:], ssps[ch][:])
                    nc.scalar.activation(nrm_c[:], ssps[ch][:], AF.Sqrt)
                    nc.vector.reciprocal(y_c[:], nrm_c[:])
                    nc.vector.tensor_tensor(t1_c[:], y_c[:], y_c[:], ALU.mult)
                    nc.vector.tensor_tensor(t1_c[:], t1_c[:], ssq_c[:], ALU.mult)
                    nc.vector.tensor_scalar(t1_c[:], t1_c[:], -0.5, 1.5, ALU.mult, ALU.add)
                    nc.vector.tensor_tensor(t1_c[:], y_c[:], t1_c[:], ALU.mult)
                    nc.vector.tensor_scalar(rn_bf[:, sl], t1_c[:], QS, None, ALU.mult)
                nc.scalar.dma_start(rn_dram[rn_slot : rn_slot + 1, :], rn_bf[:])
                rn_bc = scratch.tile([P, BLK], BF, tag="rnbc")
                nc.scalar.dma_start(
                    rn_bc[:], rn_dram[rn_slot : rn_slot + 1, :].to_broadcast((P, BLK))
                )
                for kt in range(KO):
                    nc.vector.tensor_tensor(nq_sb[:, kt, :], h_ot[kt][:], rn_bc[:], ALU.mult)

            project(lambda kt, ch: z_sb[:, kt : kt + 2, bass.ds(ch * 512, 512)], elu1, h1, n1q, 0)
            nc.scalar.dma_start(kp(ag1_in[:]), n1q[:])
            rg = [list(range(NCORES))]
            nc.gpsimd.collective_compute(
                "AllGather", ALU.bypass, replica_groups=rg,
                ins=[ag1_in[:].opt()], outs=[ag1_out[:].opt()],
            )
            # elu2 reuses the z1 slot (z1 dead after its layer 1)
            elu2 = mats.tile([P, KO, BLK], F8, tag="zt", name="elu2")
            project(lambda kt, ch: (z2a if ch == 0 else z2b)[:, kt : kt + 2, :], elu2, h2, n2q, 1)
            nc.scalar.dma_start(kp(ag2_in[:]), n2q[:])
            nc.gpsimd.collective_compute(
                "AllGather", ALU.bypass, replica_groups=rg,
                ins=[ag2_in[:].opt()], outs=[ag2_out[:].opt()],
            )

            # rowsum partials: slot 0 = diag block, slots 1.. = rotated offsets
            r11p = strip.tile([P, NT, 5], F32)
            r12p = strip.tile([P, NT, 8], F32)
            r22p = strip.tile([P, NT, 5], F32)

            def sim_iter(lhs, tt, rta, rtb, accum, cs_t=None, first=False):
                ps = psA.tile([P, 1024], F32, tag="ps_big", name="ps_sim")
                for ch, rt in ((0, rta), (1, rtb)):
                    sl = bass.ts(ch, 512)
                    for kt in range(0, KO, 2):
                        nc.tensor.matmul(
                            ps[:, sl],
                            lhs[:, kt : kt + 2, bass.ts(tt, P)],
                            rt[:, kt : kt + 2, :],
                            start=(kt == 0),
                            stop=(kt == KO - 2),
                            perf_mode=PM2,
                        )
                ex = expp.tile([P, 1024], BF, tag="ex")
                nc.scalar.activation(ex[:], ps[:], AF.Exp, scale=2.0 * IQS2, accum_out=accum)
                if cs_t is not None:
                    if first:
                        nc.vector.tensor_copy(cs_t[:], ex[:])
                    else:
                        nc.vector.tensor_tensor(cs_t[:], cs_t[:], ex[:], ALU.add)

            def colsum_flush(cs_t, row, o, nm):
                cst = scratch.tile([1, BLK], F32, tag="cst", name=f"cst_{nm}")
                for h in range(2):
                    cp = psB.tile([1, 512], F32, tag="ps_small", name=f"cp_{nm}{h}")
                    nc.tensor.matmul(
                        cp[:], ones_bf[:], cs_t[:, bass.ts(h, 512)], start=True, stop=True
                    )
                    nc.vector.tensor_copy(cst[:, bass.ts(h, 512)], cp[:])
                nc.gpsimd.dma_start(rs_in[bass.ds((pid_g + o) % 8, 1), row, :], cst[:])

            def cs_tile(nm):
                return scratch.tile([P, BLK], BF, tag="cs", bufs=3, name=f"cs_{nm}")

            # ---- diag blocks (local rhs; run while AllGathers are in flight) ----
            # S11 diag first: it only needs n1q, so it overlaps proj2's tail.
            rta_1 = n1q[:, :, 0:512]
            rtb_1 = n1q[:, :, 512:1024]
            rta_2 = n2q[:, :, 0:512]
            rtb_2 = n2q[:, :, 512:1024]
            for tt in range(NT):
                sim_iter(n1q, tt, rta_1, rtb_1, r11p[:, tt, 0:1])
            cs_d = cs_tile("d12")
            for tt in range(NT):
                sim_iter(n1q, tt, rta_2, rtb_2, r12p[:, tt, 0:1], cs_d, first=(tt == 0))
            colsum_flush(cs_d, 0, 0, "d12")
            for tt in range(NT):
                sim_iter(n2q, tt, rta_2, rtb_2, r22p[:, tt, 0:1])

            # ---- p_i = n1_i . n2_i (local diag of S12; psum holds 256*p) ----
            pps = [psB.tile([1, 512], F32, name=f"pps{_c}", tag="ps_small") for _c in range(2)]
            for kt in range(KO):
                q = scratch.tile([P, BLK], BF, tag="pq")
                nc.vector.tensor_tensor(q[:], n1q[:, kt, :], n2q[:, kt, :], ALU.mult)
                for ch in range(2):
                    nc.tensor.matmul(
                        pps[ch][:], ones_bf[:], q[:, bass.ts(ch, 512)],
                        start=(kt == 0), stop=(kt == KO - 1),
                    )
            for ch in range(2):
                p_c = small.tile([1, 512], F32, tag="ssq_c", name=f"p_c{ch}")
                nc.vector.tensor_copy(p_c[:], pps[ch][:])
                nc.gpsimd.dma_start(p_dram[ch * 512 : (ch + 1) * 512], p_c[:])

            def rhs_pair_dyn(ag, o, nm):
                idx = (pid_s + o) % 8
                a = rhsp.tile([P, KO, 512], F8, tag="rhs", name=f"ra_{nm}")
                b = rhsp.tile([P, KO, 512], F8, tag="rhs", name=f"rb_{nm}")
                blk = ag[bass.ds(idx, 1)].rearrange("one (ko ki) x -> ki (one ko) x", ki=P)
                nc.sync.dma_start(a[:], blk[:, :, 0:512])
                nc.sync.dma_start(b[:], blk[:, :, 512:1024])
                return a, b

            # ---- pass A: S11 rotated offsets 1..4 (colsums shared for 1..3) ----
            for o in (1, 2, 3, 4):
                rta, rtb = rhs_pair_dyn(ag1_out, o, f"A{o}")
                cs_t = cs_tile(f"a{o}") if o < 4 else None
                for tt in range(NT):
                    sim_iter(n1q, tt, rta, rtb, r11p[:, tt, o : o + 1], cs_t, first=(tt == 0))
                if o < 4:
                    colsum_flush(cs_t, 1, o, f"a{o}")

            # ---- pass B: S12 offsets 1..7 (+ S22 on 4..7, sharing the rhs load) ----
            for o in range(1, 8):
                rta, rtb = rhs_pair_dyn(ag2_out, o, f"B{o}")
                cs12 = cs_tile(f"b{o}")
                for tt in range(NT):
                    sim_iter(n1q, tt, rta, rtb, r12p[:, tt, o : o + 1], cs12, first=(tt == 0))
                colsum_flush(cs12, 0, o, f"b{o}")
                if o >= 4:
                    slot = o - 3  # r22p slots 1..4
                    cs22 = cs_tile(f"c{o}") if o >= 5 else None
                    for tt in range(NT):
                        sim_iter(
                            n2q, tt, rta, rtb, r22p[:, tt, slot : slot + 1],
                            cs22, first=(tt == 0),
                        )
                    if o >= 5:
                        colsum_flush(cs22, 2, o, f"c{o}")

            nc.gpsimd.collective_compute(
                "ReduceScatter", ALU.add, replica_groups=rg,
                ins=[rs_in[:].opt()], outs=[rs_out[:].opt()],
            )

            # ---------------- final loss ----------------
            r11 = small.tile([P, NT], F32, tag="r11")
            r12 = small.tile([P, NT], F32, tag="r12")
            r22 = small.tile([P, NT], F32, tag="r22")
            nc.vector.reduce_sum(r11[:], r11p[:], axis=mybir.AxisListType.X)
            nc.vector.reduce_sum(r12[:], r12p[:], axis=mybir.AxisListType.X)
            nc.vector.reduce_sum(r22[:], r22p[:], axis=mybir.AxisListType.X)
            c12 = small.tile([P, NT], F32, tag="c12")
            sh11 = small.tile([P, NT], F32, tag="sh11")
            sh22 = small.tile([P, NT], F32, tag="sh22")
            nc.sync.dma_start(c12[:], pt(rs_out[0, :]))
            nc.sync.dma_start(sh11[:], pt(rs_out[1, :]))
            nc.sync.dma_start(sh22[:], pt(rs_out[2, :]))
            p2 = small.tile([P, NT], F32, tag="p2")
            nc.sync.dma_start(p2[:], pt(p_dram[:]))

            d1 = small.tile([P, NT], F32, tag="d1")
            d2 = small.tile([P, NT], F32, tag="d2")
            nc.vector.tensor_tensor(d1[:], r11[:], sh11[:], ALU.add)
            nc.vector.tensor_tensor(d1[:], d1[:], r12[:], ALU.add)
            nc.vector.tensor_scalar(d1[:], d1[:], -E2, None, ALU.add)
            nc.vector.tensor_tensor(d2[:], r22[:], sh22[:], ALU.add)
            nc.vector.tensor_tensor(d2[:], d2[:], c12[:], ALU.add)
            nc.vector.tensor_scalar(d2[:], d2[:], -E2, None, ALU.add)
            l1 = small.tile([P, NT], F32, tag="l1")
            l2 = small.tile([P, NT], F32, tag="l2")
            nc.scalar.activation(l1[:], d1[:], AF.Ln)
            nc.scalar.activation(l2[:], d2[:], AF.Ln)
            loss = small.tile([P, NT], F32, tag="loss")
            nc.vector.tensor_tensor(loss[:], l1[:], l2[:], ALU.add)
            nc.vector.tensor_scalar(loss[:], loss[:], 0.5, None, ALU.mult)
            pm = small.tile([P, NT], F32, tag="pm")
            nc.vector.tensor_scalar(pm[:], p2[:], -2.0 * IQS2, None, ALU.mult)
            nc.vector.tensor_tensor(loss[:], loss[:], pm[:], ALU.add)
            nc.sync.dma_start(pt(out[:]), loss[:])

    nc.finalize()
    return nc


@lru_cache(maxsize=1)
def _built():
    return _build()


def _prep_inputs(z1, z2, fc1_w, fc1_b, fc2_w, fc2_b):
    f8 = ml_dtypes.float8_e4m3
    w1tq = np.ascontiguousarray(np.asarray(fc1_w, np.float32).T * WS).astype(f8)
    w2tq = np.ascontiguousarray(np.asarray(fc2_w, np.float32).T * WS).astype(f8)
    b1 = np.asarray(fc1_b, np.float32)
    # fold ELU's -1 through the *quantized* W2 so the fold is exact on device
    b2p = (
        np.asarray(fc2_b, np.float32)
        - w2tq.astype(np.float32).sum(axis=0) * (1.0 / WS)
    ).astype(np.float32)
    in_maps = []
    for c in range(NCORES):
        sl = slice(c * BLK, (c + 1) * BLK)
        in_maps.append(
            {
                "z1t": np.ascontiguousarray(np.asarray(z1[sl], np.float32).T).astype(f8),
                "z2t": np.ascontiguousarray(np.asarray(z2[sl], np.float32).T).astype(f8),
                "w1t": w1tq,
                "w2t": w2tq,
                "b1": b1,
                "b2p": b2p,
            }
        )
    return in_maps


def _install_ntff_shim():
    """Register the axon NTFF profile hook (antenv.axon_hooks is absent in
    this image; rebuild it from trn_agent_boot's ctypes recipe)."""
    import sys
    import types

    if "antenv.axon_hooks" in sys.modules:
        return True
    try:
        import antenv
        from trn_agent_boot.trn_boot import _ntff_profile_via_ctypes

        hook = _ntff_profile_via_ctypes("/opt/axon/libaxon_pjrt.so")
        if hook is None:
            return False
        m = types.ModuleType("antenv.axon_hooks")
        m._hook = hook
        m.get_axon_ntff_profile_hook = lambda: m._hook
        m.set_axon_ntff_profile_hook = lambda h: setattr(m, "_hook", h)
        sys.modules["antenv.axon_hooks"] = m
        antenv.axon_hooks = m
        # artifact upload needs egress; neuter it for local profiling
        import concourse.bass_utils as _bu

        _bu.upload_artifacts = lambda tmpdir: f"file://{tmpdir}"
        return True
    except Exception as e:
        print(f"ntff shim unavailable: {e!r}")
        return False


def _run(in_maps, trace=False):
    nc = _built()
    if trace and not _install_ntff_shim():
        trace = False
    last = None
    for attempt in range(3):
        try:
            res = run_bass_kernel_spmd(nc, in_maps, list(range(NCORES)), trace=trace)
            if all(np.isfinite(res.results[c]["out"]).all() for c in range(NCORES)):
                return res
            print("nonfinite output, retrying")
        except Exception as e:  # device occasionally wedged from a prior process
            last = e
            if "UNRECOVERABLE" not in str(e) and "UNAVAILABLE" not in str(e):
                raise
            print(f"device error (attempt {attempt}): retrying")
    if last is not None:
        raise last
    return res


def kernel(z1, z2, fc1_w, fc1_b, fc2_w, fc2_b):
    in_maps = _prep_inputs(z1, z2, fc1_w, fc1_b, fc2_w, fc2_b)
    res = _run(in_maps, trace=os.environ.get("KERNEL_TRACE", "") == "1")
    if res.exec_time_ns is not None:
        print(f"HW exec time: {res.exec_time_ns} ns")
    out = np.concatenate([res.results[c]["out"] for c in range(NCORES)])
    return out.astype(np.float32)
